# revision 46
# baseline (speedup 1.0000x reference)
"""Trainium2 Bass kernel for nn_CovarianceEstimator.

Computes, for y [B=16, R=1, A=16, T=14, S=1024] complex (given as separate
real/imag f32 tensors):
  - gather P=1024 pilot positions (sym_p, sc_p) from estimation_indices
  - per-position A x A outer products sig_p sig_p^H
  - unsorted-segment-mean over subcarrier ids sc_p
  - nearest-neighbor expand via closest_subcarrier to all S subcarriers
  - broadcast over T symbols
Output: [B, R, T, S, A, A] complex64.

Sharding: data-parallel over batch; 2 batches per core on 8 cores.

The reference's trailing broadcast_to over OFDM symbols is a zero-FLOP
replication (every t gets the same [S, A, A] covariance), so the device
computes and writes the covariance once per (batch, subcarrier) --
[B_LOC, 2(re/im), S, A*A] fp16 planes -- and the host assembles complex64
and returns a stride-0 numpy broadcast view over T.  This mirrors the
input side, where the host packs only the two pilot-symbol slabs instead
of shipping all 14 symbols to the device.

Device pipeline per batch b (all 4 even-subcarrier pairs per partition in
one merged chunk):
  psT[b][q, m, (ri a h)] = y[b, ., sym_h, 8q + 2m]      (PE transpose)
  sig tiles [q, h, m, a] = sqrt(1/2) * psT  (fp16)      (ACT, PSUM->SBUF)
  fre/fim[b][q, m, (i j)] = cov(s' = 4q + m)            (DVE)
  out[b, ri, 8q + 2m + e, :] = f*[b][q, m]              (DMA)
DVE math: products as fp16 1x ops; planar re/im output keeps every final
write dense so the h-sum adds hit the packed 2x/4x modes; the imaginary
part uses fim = R - R^T (R = K_h0 + K_h1, transposed-AP read); the
nearest-neighbor duplication e and the T broadcast are not materialised
on device (e rides a stride-0 DMA source dim, T a host broadcast view).

Two device-program builders:
  * fast path  - used when the index tensors match the PilotPattern structure
                 (meshgrid of 2 symbols x every-2nd-subcarrier, closest = even
                 floor).
  * generic    - any estimation_indices / closest_subcarrier.  Host folds the
                 whole segment-mean + NN-gather into one dense [S, P] weight
                 matrix applied on the tensor engine.
"""

import numpy as np

B, R, A, T, S = 16, 1, 16, 14, 1024
P_EST = 1024          # number of (sym, sc) estimation positions
N_CORES = 8
B_LOC = B // N_CORES  # 2 batches per core
AA2 = A * A * 2       # interleaved (re, im) row payload per subcarrier

_cache = {}


def _fast_path_info(est, closest):
    """Return (sym0, sym1) if indices match the pilot-pattern structure:
    est == meshgrid([sym0, sym1], arange(0, S, 2)) row-major and
    closest == 2*(arange(S)//2).  Else None."""
    if est.shape != (P_EST, 2) or closest.shape != (S,):
        return None
    sc = np.arange(0, S, 2, dtype=est.dtype)
    if not np.array_equal(est[: S // 2, 1], sc):
        return None
    if not np.array_equal(est[S // 2 :, 1], sc):
        return None
    sym0 = int(est[0, 0])
    sym1 = int(est[S // 2, 0])
    if not (0 <= sym0 < T and 0 <= sym1 < T):
        return None
    if sym1 <= sym0:
        return None  # keep the pilot symbols ordered; generic path covers the rest
    if not np.all(est[: S // 2, 0] == sym0):
        return None
    if not np.all(est[S // 2 :, 0] == sym1):
        return None
    if not np.array_equal(closest, (2 * (np.arange(S) // 2)).astype(closest.dtype)):
        return None
    return sym0, sym1


def _build_fast(sym0, sym1):
    import concourse.bacc as bacc
    import concourse.mybir as mybir
    from concourse.tile import TileContext

    f32 = mybir.dt.float32
    f16 = mybir.dt.float16
    S2 = S // 2  # only the even (estimated) subcarriers are shipped
    nc = bacc.Bacc(trn_type="TRN2", target_bir_lowering=False)
    # y2: host-packed pilot slabs [b, ri, a, h, s_even] (h = the 2 pilot syms)
    y2 = nc.declare_dram_parameter("y2", [B_LOC, 2, A, 2, S2], f16, isOutput=False)
    identin = nc.declare_dram_parameter("identin", [64, 64], f16, isOutput=False)
    # out is (re, im)-planar: [b, ri, s, (i j)].  Planar keeps every DVE
    # write dense step-1 (the interleaved (ij ri) layout forces 1x mode);
    # the host assembles complex64 from the two planes.
    out = nc.declare_dram_parameter("out", [B_LOC, 2, S, A * A], f16, isOutput=True)

    KS = S // 128   # 8 output subcarriers per partition
    M = KS // 2     # 4 even-subcarrier pairs per partition
    HA = 2 * A      # (h, a) / (a, h) = 32
    C = 2 * HA      # (ri, a, h) = 64 bulk partitions per batch
    SC = 0.7071067811865476  # sqrt of the segment-mean 1/2

    with TileContext(nc) as tc:
        with (
            tc.tile_pool(name="const", bufs=1) as cp,
            tc.tile_pool(name="bulk", bufs=1) as bp,
            tc.tile_pool(name="ps", bufs=1, space="PSUM") as psp,
            tc.tile_pool(name="u", bufs=2) as up,
            tc.tile_pool(name="f", bufs=1) as fp,
        ):
            # ident rides the SWDGE (gpsimd) queue so its issue overlaps the
            # HWDGE y2 issues instead of serializing on the sync sequencer.
            ident = cp.tile([C, C], f16, name="ident")
            nc.gpsimd.dma_start(out=ident[:], in_=identin[:])

            # Bulk pilot slabs: partition p = ri*32 + a*2 + h per batch,
            # each partition one contiguous 1 KB DRAM run.  Batch 0 (the
            # critical path) split over both HWDGE queues; batch 1 follows.
            bulk = [bp.tile([C, S2], f16, name=f"bulk{b}") for b in range(B_LOC)]
            for ri in range(2):
                (nc.sync, nc.scalar)[ri].dma_start(
                    out=bulk[0][ri * HA : (ri + 1) * HA],
                    in_=y2[0, ri].rearrange("a h s -> (a h) s"),
                )
            nc.sync.dma_start(
                out=bulk[1][:],
                in_=y2[1].rearrange("ri a h s -> (ri a h) s"),
            )

            psT = [
                psp.tile([128, M, C], f16, tag=f"ps{b}", name=f"ps{b}")
                for b in range(B_LOC)
            ]

            # PE transpose: estimated subcarrier s' = 4q + m into partition q
            # (all 4 m's of a batch land in one PSUM tile), then a single
            # ACT copy per (batch, re/im) moves PSUM -> SBUF as fp16 with
            # the sqrt(1/2) segment-mean factor.  One packed sig tile per
            # batch, [q, ri, h, m, a], so the re/im product pair collapses
            # into a single DVE mul.
            sgri = [
                cp.tile([128, 2, 2, M, A], f16, name=f"sgri{b}")
                for b in range(B_LOC)
            ]
            for b in range(B_LOC):
                for m in range(M):
                    nc.tensor.transpose(
                        psT[b][:, m, :],
                        bulk[b][:, m::M],
                        ident[:],
                    )
                for ri in range(2):
                    nc.scalar.mul(
                        sgri[b][:, ri],
                        psT[b][:, :, ri * HA : (ri + 1) * HA].rearrange(
                            "q m (a h) -> q h m a", a=A
                        ),
                        SC,
                    )

            # DVE outer products (fp16).  Per batch, all 4 m's in one set of
            # merged ops [q, (h m), A, A]:
            #   re: u0 = sr (x) sr, u1 = si (x) si; w = u0 + u1 (2x);
            #       fre = w[h0] + w[h1]   (dense packed write)
            #   im: K = si (x) sr; R = K[h0] + K[h1] (2x);
            #       fim = R - R^T        (transposed-AP read, 1x)
            # The nearest-neighbor duplication is NOT materialised in SBUF:
            # the output DMA fans each row out to both subcarriers with a
            # stride-0 source dim, which keeps the final DVE writes dense
            # (packed mode) and halves their size.
            fre_t = [
                fp.tile([128, M, A * A], f16, name=f"fre{b}")
                for b in range(B_LOC)
            ]
            fim_t = [
                fp.tile([128, M, A * A], f16, name=f"fim{b}")
                for b in range(B_LOC)
            ]
            full = [128, 2 * M, A, A]

            def vi(x):  # varies over i, broadcast over j
                return (
                    x.rearrange("q h n a -> q (h n) a")[:, :, :, None]
                    .to_broadcast(full)
                )

            def vj(x):  # broadcast over i, varies over j
                return (
                    x.rearrange("q h n a -> q (h n) a")[:, :, None, :]
                    .to_broadcast(full)
                )

            def vh(x, h, lo, hi):  # h-block rows n in [lo, hi), flat (i j)
                return x[:, h * M + lo : h * M + hi].rearrange(
                    "q n i j -> q n (i j)"
                )

            HM = 2 * M

            def va(x, axis):  # full sgri tile, (ri h m) flattened
                sh = [128, 2 * HM, A, A]
                v = x[:].rearrange("q r h n a -> q (r h n) a")
                return (
                    v[:, :, :, None].to_broadcast(sh)
                    if axis == 0
                    else v[:, :, None, :].to_broadcast(sh)
                )

            for b in range(B_LOC):
                uu = up.tile([128, 2 * HM, A, A], f16, tag="uu")
                kk = up.tile(full, f16, tag="kk")
                w0 = up.tile(full, f16, tag="w0")
                rr = up.tile([128, M, A, A], f16, tag="rr")
                # one mul covers both sr (x) sr and si (x) si (ri-blocks)
                nc.vector.tensor_mul(uu[:], va(sgri[b], 0), va(sgri[b], 1))
                nc.vector.tensor_mul(
                    kk[:], vi(sgri[b][:, 1]), vj(sgri[b][:, 0])
                )
                # dense h-sums hit the fp16 packed modes
                nc.vector.tensor_add(w0[:], uu[:, 0:HM], uu[:, HM : 2 * HM])
                nc.vector.tensor_add(rr[:], kk[:, 0:M], kk[:, M : 2 * M])
                # Per-half final writes + per-plane DMAs so stores overlap the
                # next half's ops; out[b, ri, 8q + 2m + e, :] = f*[b][q, m]
                # (e-fanout via a stride-0 source dim in the DMA).
                # im rides the identity fim = R - R^T (transpose distributes
                # over the h-sum).
                H2 = M // 2
                dstr = out[b, 0].rearrange(
                    "(q n e) c -> q n (e c)", q=128, n=M, e=2
                )
                dsti = out[b, 1].rearrange(
                    "(q n e) c -> q n (e c)", q=128, n=M, e=2
                )
                qa = (nc.sync, nc.scalar)
                for g in range(2):
                    lo, hi = g * H2, (g + 1) * H2
                    nc.vector.tensor_add(
                        fre_t[b][:, lo:hi], vh(w0, 0, lo, hi), vh(w0, 1, lo, hi)
                    )
                    nc.vector.tensor_sub(
                        fim_t[b][:, lo:hi].rearrange("q n (i j) -> q n i j", i=A),
                        rr[:, lo:hi],
                        rr[:, lo:hi].rearrange("q n i j -> q n j i"),
                    )
                    for n in range(lo, hi):
                        qa[(b + n) % 2].dma_start(
                            out=dstr[:, n],
                            in_=fre_t[b][:, n, None, :].to_broadcast(
                                [128, 2, A * A]
                            ),
                        )
                        qa[(b + n + 1) % 2].dma_start(
                            out=dsti[:, n],
                            in_=fim_t[b][:, n, None, :].to_broadcast(
                                [128, 2, A * A]
                            ),
                        )
    nc.finalize()
    return nc


def _build_generic(est, closest):
    """Generic program: host-gathered sig^T comes in as an input; the whole
    segment-mean + NN-gather is one dense weight matmul on the PE.
      cov[s, (i,j)] = sum_p wt[p, s] * G[p, (i,j)],  G from sig outer products.
    """
    import concourse.bacc as bacc
    import concourse.mybir as mybir
    from concourse.tile import TileContext

    f32 = mybir.dt.float32
    f16 = mybir.dt.float16
    nc = bacc.Bacc(trn_type="TRN2", target_bir_lowering=False)
    # sig^T per batch: [P_EST, A] split as [KP=8, 128, A]
    sgr = nc.declare_dram_parameter("sgr", [B_LOC, P_EST // 128, 128, A], f32, isOutput=False)
    sgi = nc.declare_dram_parameter("sgi", [B_LOC, P_EST // 128, 128, A], f32, isOutput=False)
    wt = nc.declare_dram_parameter("wt", [P_EST, S], f32, isOutput=False)
    out = nc.declare_dram_parameter("out", [B_LOC, 2, S, A * A], f16, isOutput=True)

    KP = P_EST // 128  # contraction chunks
    MS = S // 128      # output subcarrier chunks

    with TileContext(nc) as tc:
        with (
            tc.tile_pool(name="w", bufs=1) as wp,
            tc.tile_pool(name="sig", bufs=2) as sigp,
            tc.tile_pool(name="g", bufs=4) as gp,
            tc.tile_pool(name="ps", bufs=8, space="PSUM") as psp,
            tc.tile_pool(name="f", bufs=2) as fp,
        ):
            w_all = wp.tile([128, KP, S], f32, name="w_all")
            nc.sync.dma_start(
                out=w_all[:], in_=wt[:].rearrange("(k q) s -> q k s", k=KP, q=128)
            )
            for b in range(B_LOC):
                sr = sigp.tile([128, KP, A], f32, tag="sr")
                si = sigp.tile([128, KP, A], f32, tag="si")
                nc.sync.dma_start(
                    out=sr[:], in_=sgr[b].rearrange("k q a -> q k a")
                )
                nc.sync.dma_start(
                    out=si[:], in_=sgi[b].rearrange("k q a -> q k a")
                )

                f = fp.tile([128, 2, MS, A * A], f16, tag="f")
                gtiles = {}
                for k in range(KP):
                    def ii(x):
                        return x[:, k, :, None].to_broadcast([128, A, A])

                    def jj(x):
                        return x[:, k, None, :].to_broadcast([128, A, A])

                    gr = gp.tile([128, A, A], f32, tag=f"gr{k}")
                    gi = gp.tile([128, A, A], f32, tag=f"gi{k}")
                    tt = gp.tile([128, A, A], f32, tag="tt")
                    nc.vector.tensor_mul(gr[:], ii(sr), jj(sr))
                    nc.vector.tensor_mul(tt[:], ii(si), jj(si))
                    nc.vector.tensor_add(gr[:], gr[:], tt[:])
                    nc.vector.tensor_mul(gi[:], ii(si), jj(sr))
                    nc.vector.tensor_mul(tt[:], ii(sr), jj(si))
                    nc.vector.tensor_sub(gi[:], gi[:], tt[:])
                    gtiles[k] = (gr, gi)

                for m in range(MS):
                    for part in range(2):
                        pp = psp.tile([128, A * A], f32, tag="pp")
                        for k in range(KP):
                            g = gtiles[k][part]
                            nc.tensor.matmul(
                                pp[:],
                                lhsT=w_all[:, k, m * 128 : (m + 1) * 128],
                                rhs=g[:].rearrange("q i j -> q (i j)"),
                                start=(k == 0),
                                stop=(k == KP - 1),
                            )
                        nc.vector.tensor_copy(f[:, part, m, :], pp[:])

                dst = out[b].rearrange(
                    "ri (m q) ij -> q ri m ij", m=MS, q=128
                )
                nc.sync.dma_start(out=dst, in_=f[:])
    nc.finalize()
    return nc


def _get_program(est, closest):
    key = (est.tobytes(), closest.tobytes())
    hit = _cache.get(key)
    if hit is not None:
        return hit
    fast = _fast_path_info(est, closest)
    if fast is not None:
        prog = ("fast", _build_fast(*fast), None)
    else:
        counts = np.zeros(S, dtype=np.float64)
        np.add.at(counts, est[:, 1], 1.0)
        denom = np.maximum(counts, 1.0)
        # wt[p, s] = [sc_p == closest[s]] / denom[closest[s]]
        wt = (
            (est[:, 1][:, None] == closest[None, :]).astype(np.float32)
            / denom[closest][None, :].astype(np.float32)
        )
        prog = ("generic", _build_generic(est, closest), np.ascontiguousarray(wt))
    _cache[key] = prog
    return prog


_IDENT16 = np.eye(64, dtype=np.float16)


def _make_in_maps(inputs, est, kind, wt):
    yr = np.ascontiguousarray(np.asarray(inputs["y_real"], dtype=np.float32)[:, 0])
    yi = np.ascontiguousarray(np.asarray(inputs["y_imag"], dtype=np.float32)[:, 0])
    if kind == "fast":
        sym0, sym1 = int(est[0, 0]), int(est[P_EST // 2, 0])
        # pack [B, ri, a, h, s_even] fp16: the two pilot-symbol slabs at the
        # even (estimated) subcarriers only, host-gathered
        y2 = np.ascontiguousarray(
            np.stack(
                [
                    yr[:, :, (sym0, sym1), 0::2],
                    yi[:, :, (sym0, sym1), 0::2],
                ],
                axis=1,
            ).astype(np.float16)
        )
        return [
            {"y2": y2[c * B_LOC : (c + 1) * B_LOC], "identin": _IDENT16}
            for c in range(N_CORES)
        ]
    sym = est[:, 0].astype(np.int64)
    sc = est[:, 1].astype(np.int64)
    # host gather: sig[b, a, p] = y[b, a, sym_p, sc_p]
    sgr = yr[:, :, sym, sc]  # [B, A, P]
    sgi = yi[:, :, sym, sc]
    # -> [B, KP, 128, A]
    sgr = np.ascontiguousarray(
        sgr.transpose(0, 2, 1).reshape(B, P_EST // 128, 128, A)
    )
    sgi = np.ascontiguousarray(
        sgi.transpose(0, 2, 1).reshape(B, P_EST // 128, 128, A)
    )
    return [
        {
            "sgr": sgr[c * B_LOC : (c + 1) * B_LOC],
            "sgi": sgi[c * B_LOC : (c + 1) * B_LOC],
            "wt": wt,
        }
        for c in range(N_CORES)
    ]


def kernel(y_real, y_imag, estimation_indices, closest_subcarrier):
    from concourse.bass_utils import run_bass_kernel_spmd

    assert y_real.shape == (B, R, A, T, S), y_real.shape
    est = np.asarray(estimation_indices)
    closest = np.asarray(closest_subcarrier)
    kind, nc, wt = _get_program(est, closest)
    in_maps = _make_in_maps(
        {"y_real": y_real, "y_imag": y_imag}, est, kind, wt
    )

    res = run_bass_kernel_spmd(nc, in_maps, list(range(N_CORES)))
    parts = [np.asarray(res.results[c]["out"]) for c in range(N_CORES)]
    full = np.concatenate(parts, axis=0)  # [B, 2, S, A*A] fp16 (re, im planes)
    cov = np.empty((B, S, A * A), dtype=np.complex64)
    cov.real = full[:, 0]
    cov.imag = full[:, 1]
    cov = cov.reshape(B, R, 1, S, A, A)
    # The per-symbol covariance is t-independent: broadcast over T as a view.
    return np.broadcast_to(cov, (B, R, T, S, A, A))
